# revision 83
# baseline (speedup 1.0000x reference)
"""AnyprecisionLinear (w_bits=4) on 8 TRN2 NeuronCores — self-contained kernel.

kernel(x, qweight, lut) -> out
  x       (1, 2048, 8192) f32
  qweight (8192, 2048)    int32   (4x 8-bit codes per word; idx = code >> 4)
  lut     (8192, 16)      f32     (per-output-row 16-entry table)
  out     (1, 2048, 8192) f32     == einsum('bsk,ok->bso', x, lut[o, idx[o,k]])

Sharding: column-parallel — core i owns output rows [1024*i, 1024*(i+1)).
Each core gets its qweight/lut shard plus the full x (host-cast to bf16),
computes out^T (o, s); the host concatenates the 8 (1024, 2048) shards and
transposes.

Per-core kernel = software-pipelined dequant+matmul:
  - k is split into 4 round-chunks of 2048 per otile; rounds r=0..7 cover
    (group, chunk) pairs for 2 groups of 4 otiles. Dequant for round r+1
    (head = byte masks, body = selects) is issued interleaved into round r's
    matmul passes so DVE/Pool dequant hides under PE matmul work.
  - Dequant per chunk of 2048 codes (per-partition o-row tables as u32
    bf16-pair words T_m = (lut[2m], lut[2m+1])):
      head: t1w/cm/b0 byte masks (DVE tensor_scalar); full-lane mask m1 and
      byte mask m3 = is_ge(c,128) on Pool (u8 mult/is_ge conversions, the
      only Pool-legal forms).
      body: z1 = (m1 & dl1)^tb1 and z3 likewise (DVE tensor_scalar-ptr)
      preselect T2/T3 and T6/T7 by bit5; the custom DVE op APQ_SEL4_ANT
      (registered at import) then does each half-table's 4-way select in ONE
      instruction: out = select(cm>=64, z, select(cm in {32,96}, T1, T0))
      with cm = (code & 0x60) bytes auto-converted u8->f32 by the read
      stage; copy_predicated merges halves on bit7 (m3) and resolves the
      u16 lo/hi pick on bit4 (b0), quartered for early transpose release.
  - Weights are PE-transposed (128x128) into per-(slot, round) Wt tiles.
  - Matmuls accumulate 16-ktile segments per (otile, 512-token sb) in PSUM;
    segments combine across rounds in SBUF f32 accumulators (Act copies
    PSUM->SBUF, Pool tensor_tensor adds); sliced out-DMAs at group ends.
  - x^T tiles stream via hardware transpose-DMAs from the host-cast bf16 x,
    prefetched two passes ahead; all HWDGE DMAs ride the SP queue so the
    global HWDGE completion-sem ring never serializes across queues.
"""
import numpy as np
import ml_dtypes

import concourse.mybir as mybir
from concourse import bacc, bass_utils
from concourse.tile import TileContext
from concourse.masks import make_identity

dt = mybir.dt
A = mybir.AluOpType

O, K, S = 1024, 8192, 2048    # per-core out rows, contraction, tokens
P = 128
KC = 2048                     # codes per dequant chunk (k span per round)
NCH = K // KC                 # rounds (chunk index) per otile = 4
SEG = KC // P                 # ktiles per segment = 16
SBW = 512                     # tokens per psum segment accumulator
NSB = S // SBW                # 4 sb passes per round
GN = 4                        # otiles per group
NG = O // P // GN             # 2 groups
NR = NCH * NG                 # 8 global rounds
N_CORES = 8


def _host_tables(lut_shard):
    U = lut_shard.astype(ml_dtypes.bfloat16).view(np.uint16).astype(np.uint32)
    T = U[:, 0::2] | (U[:, 1::2] << 16)
    tb = T[:, 0::2].copy()
    dl = (T[:, 0::2] ^ T[:, 1::2]).copy()
    tw = np.ascontiguousarray(T[:, [0, 1, 4, 5]])
    return tb, dl, tw


_SEL4 = None


def _register_sel4():
    """Custom DVE op: 4-way table select keyed on (bit6, bit5) of the code,
    delivered as in0 bytes (code & 0x60) -> {0, 32, 64, 96} after the read
    stage's u8->f32 conversion.
      out = in1            if in0 >= 64        (bit6 set: upper pair, preselected)
            s1             elif in0 in {32,96} (bit5 set)
            s0             else
    """
    global _SEL4
    if _SEL4 is not None:
        return _SEL4
    from concourse import dve_ops as dve_ops_mod
    from concourse.dve_spec import Spec, Src0, Src1, C0, C1, C2, Zero, select, ne, lower
    from concourse.dve_uop import DveOpSpec

    name = "APQ_SEL4_ANT"
    for o in dve_ops_mod.OPS:
        if o.name == name:
            _SEL4 = o
            return o
    body = select(Src0 >= C2, Src1, select(ne(Src0, Zero) & ne(Src0, C2), C1, C0))

    def ref(in0, in1, s0, s1, imm2):
        a = in0.astype(np.float32)
        inner = np.where((a != 0) & (a != imm2), s1, s0)
        return np.where(a >= imm2, in1, inner).astype(np.float32)

    spec = Spec(body=body, reference=ref)
    row = max(dve_ops_mod._SUB_OPCODE_FOR_NAME.values()) + 1
    dve_ops_mod._SUB_OPCODE_FOR_NAME[name] = row
    shas = {}
    for ver in ("v3", "v4"):
        uops = lower(spec, ver=ver)
        shas[ver] = DveOpSpec(name=name, opcode=row, uops=uops, rd1_en=True).sha(ver)
    op = dve_ops_mod.DveOp(name, spec, subdim=False, uops_sha=shas)
    dve_ops_mod.OPS.append(op)
    dve_ops_mod.CUSTOM_DVE_SPECS[name] = spec
    _SEL4 = op
    return op


def _build_kernel(nc):
    xbf_in = nc.declare_dram_parameter("xbf", [S, K], dt.bfloat16, isOutput=False)
    qw_in = nc.declare_dram_parameter("qw", [O, K], dt.uint8, isOutput=False)
    tb_in = nc.declare_dram_parameter("tb", [O, 4], dt.uint32, isOutput=False)
    dl_in = nc.declare_dram_parameter("dl", [O, 4], dt.uint32, isOutput=False)
    tw_in = nc.declare_dram_parameter("tw", [O, 4], dt.uint32, isOutput=False)
    sel4 = _register_sel4()
    out_d = nc.declare_dram_parameter("out", [O, S], dt.float32, isOutput=True)

    with TileContext(nc) as tc:
        with tc.tile_pool(name="const", bufs=1) as cpool, \
             tc.tile_pool(name="tabs", bufs=1) as tabpool, \
             tc.tile_pool(name="wt", bufs=1) as wtpool, \
             tc.tile_pool(name="sacc", bufs=1) as saccpool, \
             tc.tile_pool(name="deq", bufs=1) as dqpool, \
             tc.tile_pool(name="xt", bufs=1) as xtpool, \
             tc.tile_pool(name="pst", bufs=1, space="PSUM") as pst, \
             tc.tile_pool(name="psacc", bufs=1, space="PSUM") as psacc:

            ident = cpool.tile([P, P], dt.bfloat16, name="ident")
            idf = cpool.tile([P, P], dt.float32, name="idf")
            make_identity(nc, idf)
            nc.vector.tensor_copy(out=ident, in_=idf)

            NOT = O // P
            dqpool_early = dqpool  # alias for clarity: qw0 prefetch precedes tabs
            qw0_pre = dqpool_early.tile([P, KC], dt.uint8, name="qw", tag="qw", bufs=2)
            nc.sync.dma_start(out=qw0_pre, in_=qw_in[0:P, 0:KC])

            tb_all = tabpool.tile([P, 4 * NOT], dt.uint32, name="tb_all")
            dl_all = tabpool.tile([P, 4 * NOT], dt.uint32, name="dl_all")
            nc.sync.dma_start(out=tb_all.rearrange("p (g t) -> p g t", t=4),
                              in_=tb_in.rearrange("(g p) t -> p g t", p=P))
            nc.sync.dma_start(out=dl_all.rearrange("p (g t) -> p g t", t=4),
                              in_=dl_in.rearrange("(g p) t -> p g t", p=P))
            tw_all = tabpool.tile([P, 4 * NOT], dt.uint32, name="tw_all")
            nc.sync.dma_start(out=tw_all.rearrange("p (g t) -> p g t", t=4),
                              in_=tw_in.rearrange("(g p) t -> p g t", p=P))
            tabs = [(tb_all[:, 4 * ot:4 * (ot + 1)], dl_all[:, 4 * ot:4 * (ot + 1)],
                     tw_all[:, 4 * ot:4 * (ot + 1)])
                    for ot in range(NOT)]

            wts = {}          # (slot, c) -> Wt tile [P, KC] (k on partitions)
            pending = {}      # slot -> (z0, c) awaiting transpose
            xt_tiles = {}     # (sb, j) -> xt tile [P, SBW]
            saccs = {}        # slot -> SBUF accumulator [P, S]

            heads = {}   # slot -> state from deq_head awaiting deq_body

            def deq_head(r, slot, pre_qw=None, m1_dve=False):
                """Mask/byte prep: feeds Pool (m1/m3) as early as possible."""
                g, c = divmod(r, NCH)
                ot = g * GN + slot
                if pre_qw is not None:
                    qw = pre_qw
                else:
                    qw = dqpool.tile([P, KC], dt.uint8, name="qw", tag="qw", bufs=2)
                    nc.sync.dma_start(out=qw,
                                      in_=qw_in[ot * P:(ot + 1) * P, c * KC:(c + 1) * KC])
                cw = qw.bitcast(dt.uint32)
                t1w = dqpool.tile([P, KC // 4], dt.uint32, name="t1w", tag="t1w", bufs=2)
                nc.vector.tensor_scalar(out=t1w, in0=cw, scalar1=5, scalar2=0x01010101,
                                        op0=A.logical_shift_right, op1=A.bitwise_and)
                m1 = dqpool.tile([P, KC], dt.int32, name="m1", tag="m1", bufs=2)
                m1_eng = nc.vector if m1_dve else nc.gpsimd
                m1_eng.tensor_scalar(out=m1, in0=t1w.bitcast(dt.uint8), scalar1=-1.0,
                                     scalar2=None, op0=A.mult)
                m3 = dqpool.tile([P, KC], dt.uint8, name="m3", tag="m3", bufs=2)
                nc.gpsimd.tensor_scalar(out=m3, in0=qw, scalar1=128.0, scalar2=None,
                                        op0=A.is_ge)
                cm = dqpool.tile([P, KC // 4], dt.uint32, name="cm", tag="cm", bufs=2)
                nc.vector.tensor_scalar(out=cm, in0=cw, scalar1=0x60606060, scalar2=None,
                                        op0=A.bitwise_and)
                b0 = dqpool.tile([P, KC // 4], dt.uint32, name="b0", tag="b0", bufs=2)
                nc.vector.tensor_scalar(out=b0, in0=cw, scalar1=0x10101010, scalar2=None,
                                        op0=A.bitwise_and)
                heads[slot] = (r, m1, m3, cm, b0)

            def deq_body(slot, nsplit=1):
                r, m1, m3, cm, b0 = heads.pop(slot)
                g, c = divmod(r, NCH)
                ot = g * GN + slot
                tb_sb, dl_sb, tw_sb = tabs[ot]
                z13 = dqpool.tile([P, KC], dt.uint32, name="z13", tag="z13", bufs=1)
                zA = dqpool.tile([P, KC], dt.uint32, name="zA", tag="zA", bufs=2)
                zB = dqpool.tile([P, KC], dt.uint32, name="zB", tag="zB", bufs=1)
                L = KC // nsplit
                W = L // 4
                for s in range(nsplit):
                    kb, wb = s * L, s * W
                    m1s = m1[:, kb:kb + L]
                    cms = cm[:, wb:wb + W]
                    z13s = z13[:, kb:kb + L]
                    nc.vector.tensor_scalar(out=z13s, in0=m1s.bitcast(dt.uint32),
                                            scalar1=dl_sb[:, 1:2], scalar2=tb_sb[:, 1:2],
                                            op0=A.bitwise_and, op1=A.bitwise_xor)
                    zAs = zA[:, kb:kb + L]
                    nc.vector._custom_dve(
                        sel4, out=zAs.bitcast(dt.float32), in0=cms.bitcast(dt.uint8),
                        in1=z13s.bitcast(dt.float32),
                        s0=tw_sb[:, 0:1].bitcast(dt.float32),
                        s1=tw_sb[:, 1:2].bitcast(dt.float32), imm2=64.0)
                    nc.vector.tensor_scalar(out=z13s, in0=m1s.bitcast(dt.uint32),
                                            scalar1=dl_sb[:, 3:4], scalar2=tb_sb[:, 3:4],
                                            op0=A.bitwise_and, op1=A.bitwise_xor)
                    zBs = zB[:, kb:kb + L]
                    nc.vector._custom_dve(
                        sel4, out=zBs.bitcast(dt.float32), in0=cms.bitcast(dt.uint8),
                        in1=z13s.bitcast(dt.float32),
                        s0=tw_sb[:, 2:3].bitcast(dt.float32),
                        s1=tw_sb[:, 3:4].bitcast(dt.float32), imm2=64.0)
                    nc.vector.copy_predicated(out=zAs, mask=m3[:, kb:kb + L], data=zBs)
                    zv = zAs.bitcast(dt.uint16).rearrange("p (k two) -> p k two", two=2)
                    b0v = b0[:, wb:wb + W].bitcast(dt.uint8).rearrange(
                        "p (q k) -> p q k", q=4 // nsplit)
                    Q = L // (4 // nsplit)
                    for q in range(4 // nsplit):
                        nc.vector.copy_predicated(out=zv[:, q * Q:(q + 1) * Q, 0],
                                                  mask=b0v[:, q, :],
                                                  data=zv[:, q * Q:(q + 1) * Q, 1])
                pending[slot] = (zA, c)

            def deq_compute(r, slot, nsplit=1, pre_qw=None):
                deq_head(r, slot, pre_qw=pre_qw)
                deq_body(slot, nsplit=nsplit)

            def deq_finish(slot):
                z0, c = pending.pop(slot)
                wt = wtpool.tile([P, KC], dt.bfloat16, name=f"w{slot}_{c}",
                                 tag=f"w{slot}_{c}", bufs=1)
                wch = z0.bitcast(dt.bfloat16).rearrange("p (k two) -> p k two", two=2)[:, :, 0]
                for jg in range(SEG // 4):
                    pt = pst.tile([P, 4 * P], dt.bfloat16, name="pt", tag="pt", bufs=2)
                    for j4 in range(4):
                        j = jg * 4 + j4
                        nc.tensor.transpose(pt[:, j4 * P:(j4 + 1) * P],
                                            wch[:, j * P:(j + 1) * P], ident)
                    nc.scalar.copy(out=wt[:, jg * 4 * P:(jg + 1) * 4 * P], in_=pt)
                wts[(slot, c)] = wt

            def issue_xt(r, sbp):
                # one DMA covers a PAIR of 512-token passes: in [1024,128] -> out [128,1024]
                c = r % NCH
                for j in range(SEG):
                    kt = c * SEG + j
                    xt = xtpool.tile([P, 2 * SBW], dt.bfloat16, name="xt", tag="xt", bufs=20)
                    nc.sync.dma_start_transpose(
                        out=xt,
                        in_=xbf_in[sbp * 2 * SBW:(sbp + 1) * 2 * SBW, kt * P:(kt + 1) * P])
                    xt_tiles[(sbp, j)] = xt

            def mm_block(r, sb, slot):
                c = r % NCH
                acc = psacc.tile([P, SBW], dt.float32, name="acc", tag="acc", bufs=6)
                wt = wts[(slot, c)]
                for j in range(SEG):
                    nc.tensor.matmul(acc, wt[:, j * P:(j + 1) * P],
                                     xt_tiles[(sb // 2, j)][:, (sb % 2) * SBW:(sb % 2 + 1) * SBW],
                                     start=(j == 0), stop=(j == SEG - 1))
                dst = saccs[slot][:, sb * SBW:(sb + 1) * SBW]
                if c == 0:
                    nc.scalar.copy(out=dst, in_=acc)
                else:
                    tmp = dqpool.tile([P, SBW], dt.float32, name="tmp", tag="tmp", bufs=1)
                    nc.scalar.copy(out=tmp, in_=acc)
                    nc.gpsimd.tensor_tensor(out=dst, in0=tmp, in1=dst, op=A.add)

            def mm_pass(r, sb):
                for slot in range(GN):
                    mm_block(r, sb, slot)

            # prologue: round 0 dequant; diagonal mm issue so PE follows the
            # DVE chunk stream slot-by-slot instead of waiting for all four.
            deq_head(0, 0, pre_qw=qw0_pre)
            deq_head(0, 1)
            deq_body(0, nsplit=2)
            deq_head(0, 2)
            deq_body(1)
            deq_head(0, 3)
            deq_body(2)
            deq_body(3)
            issue_xt(0, 0)
            for slot in range(GN):
                saccs[slot] = saccpool.tile([P, S], dt.float32,
                                            name=f"sacc{slot}", tag=f"sacc{slot}", bufs=1)
            for slot in range(GN):
                deq_finish(slot)
                mm_block(0, 0, slot)
                mm_block(0, 1, slot)

            for r in range(NR):
                g, c = divmod(r, NCH)
                if c == 0 and r > 0:
                    for slot in range(GN):
                        saccs[slot] = saccpool.tile([P, S], dt.float32,
                                                    name=f"sacc{slot}", tag=f"sacc{slot}", bufs=1)
                for sb in range(NSB):
                    if sb == 0:
                        issue_xt(r, 1)
                    elif sb == 2 and r < NR - 1:
                        issue_xt(r + 1, 0)
                    if not (r == 0 and sb < 2):
                        # passes (0,0)/(0,1) were issued in the diagonal prologue
                        mm_pass(r, sb)
                    if r < NR - 1:
                        deq_head(r + 1, sb)
                        if sb >= 1:
                            deq_body(sb - 1)
                            deq_finish(sb - 1)
                        if sb == NSB - 1:
                            deq_body(3)
                            deq_finish(3)
                if c == NCH - 1:
                    for sb in range(NSB):
                        for slot in range(GN):
                            ot = g * GN + slot
                            nc.sync.dma_start(
                                out=out_d[ot * P:(ot + 1) * P, sb * SBW:(sb + 1) * SBW],
                                in_=saccs[slot][:, sb * SBW:(sb + 1) * SBW])


_NC_CACHE = None


def _get_nc():
    global _NC_CACHE
    if _NC_CACHE is None:
        nc = bacc.Bacc("TRN2", num_devices=N_CORES)
        _build_kernel(nc)
        nc.compile()
        _NC_CACHE = nc
    return _NC_CACHE


def kernel(x, qweight, lut):
    x = np.asarray(x)
    qweight = np.asarray(qweight)
    lut = np.asarray(lut)
    xbf = np.ascontiguousarray(
        x.reshape(S, K).astype(np.float32, copy=False)).astype(ml_dtypes.bfloat16)

    in_maps = []
    for c in range(N_CORES):
        o0, o1 = c * O, (c + 1) * O
        qb = np.ascontiguousarray(qweight[o0:o1]).view(np.uint8).reshape(O, K)
        tb, dl, tw = _host_tables(lut[o0:o1])
        in_maps.append({"xbf": xbf, "qw": qb, "tb": tb, "dl": dl, "tw": tw})

    nc = _get_nc()
    res = bass_utils.run_bass_kernel_spmd(nc, in_maps, core_ids=list(range(N_CORES)))
    out_full = np.concatenate([res.results[c]["out"] for c in range(N_CORES)], axis=0)
    return np.ascontiguousarray(out_full.T).reshape(1, S, 8192).astype(np.float32, copy=False)


# revision 84
# speedup vs baseline: 1.0190x; 1.0190x over previous
"""AnyprecisionLinear (w_bits=4) on 8 TRN2 NeuronCores — self-contained kernel.

kernel(x, qweight, lut) -> out
  x       (1, 2048, 8192) f32
  qweight (8192, 2048)    int32   (4x 8-bit codes per word; idx = code >> 4)
  lut     (8192, 16)      f32     (per-output-row 16-entry table)
  out     (1, 2048, 8192) f32     == einsum('bsk,ok->bso', x, lut[o, idx[o,k]])

Sharding: column-parallel — core i owns output rows [1024*i, 1024*(i+1)).
Each core gets its qweight/lut shard plus the full x (host-cast to bf16),
computes out^T (o, s); the host concatenates the 8 (1024, 2048) shards and
transposes.

Per-core kernel = software-pipelined dequant+matmul:
  - k is split into 4 round-chunks of 2048 per otile; rounds r=0..7 cover
    (group, chunk) pairs for 2 groups of 4 otiles. Dequant for round r+1
    (head = byte masks, body = selects) is issued interleaved into round r's
    matmul passes so DVE/Pool dequant hides under PE matmul work.
  - Dequant per chunk of 2048 codes (per-partition o-row tables as u32
    bf16-pair words T_m = (lut[2m], lut[2m+1])):
      head: t1w/cm/b0 byte masks (DVE tensor_scalar); full-lane mask m1 and
      byte mask m3 = is_ge(c,128) on Pool (u8 mult/is_ge conversions, the
      only Pool-legal forms).
      body: z1 = (m1 & dl1)^tb1 and z3 likewise (DVE tensor_scalar-ptr)
      preselect T2/T3 and T6/T7 by bit5; the custom DVE op APQ_SEL4_ANT
      (registered at import) then does each half-table's 4-way select in ONE
      instruction: out = select(cm>=64, z, select(cm in {32,96}, T1, T0))
      with cm = (code & 0x60) bytes auto-converted u8->f32 by the read
      stage; copy_predicated merges halves on bit7 (m3) and resolves the
      u16 lo/hi pick on bit4 (b0), quartered for early transpose release.
  - Weights are PE-transposed (128x128) into per-(slot, round) Wt tiles.
  - Matmuls accumulate 16-ktile segments per (otile, 512-token sb) in PSUM;
    segments combine across rounds in SBUF f32 accumulators (Act copies
    PSUM->SBUF, Pool tensor_tensor adds); sliced out-DMAs at group ends.
  - x^T tiles stream via hardware transpose-DMAs from the host-cast bf16 x,
    prefetched two passes ahead; all HWDGE DMAs ride the SP queue so the
    global HWDGE completion-sem ring never serializes across queues.
"""
import numpy as np
import ml_dtypes

import concourse.mybir as mybir
from concourse import bacc, bass_utils
from concourse.tile import TileContext
from concourse.masks import make_identity

dt = mybir.dt
A = mybir.AluOpType

O, K, S = 1024, 8192, 2048    # per-core out rows, contraction, tokens
P = 128
KC = 2048                     # codes per dequant chunk (k span per round)
NCH = K // KC                 # rounds (chunk index) per otile = 4
SEG = KC // P                 # ktiles per segment = 16
SBW = 512                     # tokens per psum segment accumulator
NSB = S // SBW                # 4 sb passes per round
GN = 4                        # otiles per group
NG = O // P // GN             # 2 groups
NR = NCH * NG                 # 8 global rounds
N_CORES = 8


def _host_tables(lut_shard):
    U = lut_shard.astype(ml_dtypes.bfloat16).view(np.uint16).astype(np.uint32)
    T = U[:, 0::2] | (U[:, 1::2] << 16)
    tb = T[:, 0::2].copy()
    dl = (T[:, 0::2] ^ T[:, 1::2]).copy()
    tw = np.ascontiguousarray(T[:, [0, 1, 4, 5]])
    return tb, dl, tw


_SEL4 = None


def _register_sel4():
    """Custom DVE op: 4-way table select keyed on (bit6, bit5) of the code,
    delivered as in0 bytes (code & 0x60) -> {0, 32, 64, 96} after the read
    stage's u8->f32 conversion.
      out = in1            if in0 >= 64        (bit6 set: upper pair, preselected)
            s1             elif in0 in {32,96} (bit5 set)
            s0             else
    """
    global _SEL4
    if _SEL4 is not None:
        return _SEL4
    from concourse import dve_ops as dve_ops_mod
    from concourse.dve_spec import Spec, Src0, Src1, C0, C1, C2, Zero, select, ne, lower
    from concourse.dve_uop import DveOpSpec

    name = "APQ_SEL4_ANT"
    for o in dve_ops_mod.OPS:
        if o.name == name:
            _SEL4 = o
            return o
    body = select(Src0 >= C2, Src1, select(ne(Src0, Zero) & ne(Src0, C2), C1, C0))

    def ref(in0, in1, s0, s1, imm2):
        a = in0.astype(np.float32)
        inner = np.where((a != 0) & (a != imm2), s1, s0)
        return np.where(a >= imm2, in1, inner).astype(np.float32)

    spec = Spec(body=body, reference=ref)
    row = max(dve_ops_mod._SUB_OPCODE_FOR_NAME.values()) + 1
    dve_ops_mod._SUB_OPCODE_FOR_NAME[name] = row
    shas = {}
    for ver in ("v3", "v4"):
        uops = lower(spec, ver=ver)
        shas[ver] = DveOpSpec(name=name, opcode=row, uops=uops, rd1_en=True).sha(ver)
    op = dve_ops_mod.DveOp(name, spec, subdim=False, uops_sha=shas)
    dve_ops_mod.OPS.append(op)
    dve_ops_mod.CUSTOM_DVE_SPECS[name] = spec
    _SEL4 = op
    return op


def _build_kernel(nc):
    xbf_in = nc.declare_dram_parameter("xbf", [S, K], dt.bfloat16, isOutput=False)
    qw_in = nc.declare_dram_parameter("qw", [O, K], dt.uint8, isOutput=False)
    tb_in = nc.declare_dram_parameter("tb", [O, 4], dt.uint32, isOutput=False)
    dl_in = nc.declare_dram_parameter("dl", [O, 4], dt.uint32, isOutput=False)
    tw_in = nc.declare_dram_parameter("tw", [O, 4], dt.uint32, isOutput=False)
    sel4 = _register_sel4()
    out_d = nc.declare_dram_parameter("out", [O, S], dt.float32, isOutput=True)

    with TileContext(nc) as tc:
        with tc.tile_pool(name="const", bufs=1) as cpool, \
             tc.tile_pool(name="tabs", bufs=1) as tabpool, \
             tc.tile_pool(name="wt", bufs=1) as wtpool, \
             tc.tile_pool(name="sacc", bufs=1) as saccpool, \
             tc.tile_pool(name="deq", bufs=1) as dqpool, \
             tc.tile_pool(name="xt", bufs=1) as xtpool, \
             tc.tile_pool(name="pst", bufs=1, space="PSUM") as pst, \
             tc.tile_pool(name="psacc", bufs=1, space="PSUM") as psacc:

            ident = cpool.tile([P, P], dt.bfloat16, name="ident")
            idf = cpool.tile([P, P], dt.float32, name="idf")
            make_identity(nc, idf)
            nc.vector.tensor_copy(out=ident, in_=idf)

            NOT = O // P
            dqpool_early = dqpool  # alias for clarity: qw0 prefetch precedes tabs
            qw0_pre = dqpool_early.tile([P, KC], dt.uint8, name="qw", tag="qw", bufs=2)
            nc.sync.dma_start(out=qw0_pre, in_=qw_in[0:P, 0:KC])

            tb_all = tabpool.tile([P, 4 * NOT], dt.uint32, name="tb_all")
            dl_all = tabpool.tile([P, 4 * NOT], dt.uint32, name="dl_all")
            nc.sync.dma_start(out=tb_all.rearrange("p (g t) -> p g t", t=4),
                              in_=tb_in.rearrange("(g p) t -> p g t", p=P))
            nc.sync.dma_start(out=dl_all.rearrange("p (g t) -> p g t", t=4),
                              in_=dl_in.rearrange("(g p) t -> p g t", p=P))
            tw_all = tabpool.tile([P, 4 * NOT], dt.uint32, name="tw_all")
            nc.sync.dma_start(out=tw_all.rearrange("p (g t) -> p g t", t=4),
                              in_=tw_in.rearrange("(g p) t -> p g t", p=P))
            tabs = [(tb_all[:, 4 * ot:4 * (ot + 1)], dl_all[:, 4 * ot:4 * (ot + 1)],
                     tw_all[:, 4 * ot:4 * (ot + 1)])
                    for ot in range(NOT)]

            wts = {}          # (slot, c) -> Wt tile [P, KC] (k on partitions)
            pending = {}      # slot -> (z0, c) awaiting transpose
            xt_tiles = {}     # (sb, j) -> xt tile [P, SBW]
            saccs = {}        # slot -> SBUF accumulator [P, S]

            heads = {}   # slot -> state from deq_head awaiting deq_body

            def deq_head(r, slot, pre_qw=None, m1_dve=False):
                """Mask/byte prep: feeds Pool (m1/m3) as early as possible."""
                g, c = divmod(r, NCH)
                ot = g * GN + slot
                if pre_qw is not None:
                    qw = pre_qw
                else:
                    qw = dqpool.tile([P, KC], dt.uint8, name="qw", tag="qw", bufs=2)
                    nc.sync.dma_start(out=qw,
                                      in_=qw_in[ot * P:(ot + 1) * P, c * KC:(c + 1) * KC])
                cw = qw.bitcast(dt.uint32)
                t1w = dqpool.tile([P, KC // 4], dt.uint32, name="t1w", tag="t1w", bufs=2)
                nc.vector.tensor_scalar(out=t1w, in0=cw, scalar1=5, scalar2=0x01010101,
                                        op0=A.logical_shift_right, op1=A.bitwise_and)
                m1 = dqpool.tile([P, KC], dt.int32, name="m1", tag="m1", bufs=2)
                m1_eng = nc.vector if m1_dve else nc.gpsimd
                m1_eng.tensor_scalar(out=m1, in0=t1w.bitcast(dt.uint8), scalar1=-1.0,
                                     scalar2=None, op0=A.mult)
                m3 = dqpool.tile([P, KC], dt.uint8, name="m3", tag="m3", bufs=2)
                nc.gpsimd.tensor_scalar(out=m3, in0=qw, scalar1=128.0, scalar2=None,
                                        op0=A.is_ge)
                cm = dqpool.tile([P, KC // 4], dt.uint32, name="cm", tag="cm", bufs=2)
                nc.vector.tensor_scalar(out=cm, in0=cw, scalar1=0x60606060, scalar2=None,
                                        op0=A.bitwise_and)
                b0 = dqpool.tile([P, KC // 4], dt.uint32, name="b0", tag="b0", bufs=2)
                nc.vector.tensor_scalar(out=b0, in0=cw, scalar1=0x10101010, scalar2=None,
                                        op0=A.bitwise_and)
                heads[slot] = (r, m1, m3, cm, b0)

            def deq_body(slot, nsplit=1):
                r, m1, m3, cm, b0 = heads.pop(slot)
                g, c = divmod(r, NCH)
                ot = g * GN + slot
                tb_sb, dl_sb, tw_sb = tabs[ot]
                z13 = dqpool.tile([P, KC], dt.uint32, name="z13", tag="z13", bufs=1)
                zA = dqpool.tile([P, KC], dt.uint32, name="zA", tag="zA", bufs=2)
                zB = dqpool.tile([P, KC], dt.uint32, name="zB", tag="zB", bufs=1)
                L = KC // nsplit
                W = L // 4
                for s in range(nsplit):
                    kb, wb = s * L, s * W
                    m1s = m1[:, kb:kb + L]
                    cms = cm[:, wb:wb + W]
                    z13s = z13[:, kb:kb + L]
                    nc.vector.tensor_scalar(out=z13s, in0=m1s.bitcast(dt.uint32),
                                            scalar1=dl_sb[:, 1:2], scalar2=tb_sb[:, 1:2],
                                            op0=A.bitwise_and, op1=A.bitwise_xor)
                    zAs = zA[:, kb:kb + L]
                    nc.vector._custom_dve(
                        sel4, out=zAs.bitcast(dt.float32), in0=cms.bitcast(dt.uint8),
                        in1=z13s.bitcast(dt.float32),
                        s0=tw_sb[:, 0:1].bitcast(dt.float32),
                        s1=tw_sb[:, 1:2].bitcast(dt.float32), imm2=64.0)
                    nc.vector.tensor_scalar(out=z13s, in0=m1s.bitcast(dt.uint32),
                                            scalar1=dl_sb[:, 3:4], scalar2=tb_sb[:, 3:4],
                                            op0=A.bitwise_and, op1=A.bitwise_xor)
                    zBs = zB[:, kb:kb + L]
                    nc.vector._custom_dve(
                        sel4, out=zBs.bitcast(dt.float32), in0=cms.bitcast(dt.uint8),
                        in1=z13s.bitcast(dt.float32),
                        s0=tw_sb[:, 2:3].bitcast(dt.float32),
                        s1=tw_sb[:, 3:4].bitcast(dt.float32), imm2=64.0)
                    nc.vector.copy_predicated(out=zAs, mask=m3[:, kb:kb + L], data=zBs)
                    zv = zAs.bitcast(dt.uint16).rearrange("p (k two) -> p k two", two=2)
                    b0v = b0[:, wb:wb + W].bitcast(dt.uint8).rearrange(
                        "p (q k) -> p q k", q=4 // nsplit)
                    Q = L // (4 // nsplit)
                    for q in range(4 // nsplit):
                        nc.vector.copy_predicated(out=zv[:, q * Q:(q + 1) * Q, 0],
                                                  mask=b0v[:, q, :],
                                                  data=zv[:, q * Q:(q + 1) * Q, 1])
                pending[slot] = (zA, c)

            def deq_compute(r, slot, nsplit=1, pre_qw=None):
                deq_head(r, slot, pre_qw=pre_qw)
                deq_body(slot, nsplit=nsplit)

            def deq_finish(slot):
                z0, c = pending.pop(slot)
                wt = wtpool.tile([P, KC], dt.bfloat16, name=f"w{slot}_{c}",
                                 tag=f"w{slot}_{c}", bufs=1)
                wch = z0.bitcast(dt.bfloat16).rearrange("p (k two) -> p k two", two=2)[:, :, 0]
                for jg in range(SEG // 4):
                    pt = pst.tile([P, 4 * P], dt.bfloat16, name="pt", tag="pt", bufs=2)
                    for j4 in range(4):
                        j = jg * 4 + j4
                        nc.tensor.transpose(pt[:, j4 * P:(j4 + 1) * P],
                                            wch[:, j * P:(j + 1) * P], ident)
                    nc.scalar.copy(out=wt[:, jg * 4 * P:(jg + 1) * 4 * P], in_=pt)
                wts[(slot, c)] = wt

            def issue_xt(r, sbp):
                # one DMA covers a PAIR of 512-token passes: in [1024,128] -> out [128,1024]
                c = r % NCH
                for j in range(SEG):
                    kt = c * SEG + j
                    xt = xtpool.tile([P, 2 * SBW], dt.bfloat16, name="xt", tag="xt", bufs=20)
                    nc.sync.dma_start_transpose(
                        out=xt,
                        in_=xbf_in[sbp * 2 * SBW:(sbp + 1) * 2 * SBW, kt * P:(kt + 1) * P])
                    xt_tiles[(sbp, j)] = xt

            def mm_block(r, sb, slot):
                c = r % NCH
                acc = psacc.tile([P, SBW], dt.float32, name="acc", tag="acc", bufs=6)
                wt = wts[(slot, c)]
                for j in range(SEG):
                    nc.tensor.matmul(acc, wt[:, j * P:(j + 1) * P],
                                     xt_tiles[(sb // 2, j)][:, (sb % 2) * SBW:(sb % 2 + 1) * SBW],
                                     start=(j == 0), stop=(j == SEG - 1))
                dst = saccs[slot][:, sb * SBW:(sb + 1) * SBW]
                if c == 0:
                    nc.scalar.copy(out=dst, in_=acc)
                else:
                    tmp = dqpool.tile([P, SBW], dt.float32, name="tmp", tag="tmp", bufs=1)
                    nc.scalar.copy(out=tmp, in_=acc)
                    nc.gpsimd.tensor_tensor(out=dst, in0=tmp, in1=dst, op=A.add)

            def mm_pass(r, sb):
                for slot in range(GN):
                    mm_block(r, sb, slot)

            # prologue: round 0 dequant; diagonal mm issue so PE follows the
            # DVE chunk stream slot-by-slot instead of waiting for all four.
            deq_head(0, 0, pre_qw=qw0_pre)
            deq_head(0, 1)
            deq_body(0, nsplit=2)
            deq_head(0, 2)
            deq_body(1, nsplit=2)
            deq_head(0, 3)
            deq_body(2, nsplit=2)
            deq_body(3, nsplit=2)
            issue_xt(0, 0)
            for slot in range(GN):
                saccs[slot] = saccpool.tile([P, S], dt.float32,
                                            name=f"sacc{slot}", tag=f"sacc{slot}", bufs=1)
            for slot in range(GN):
                deq_finish(slot)
                mm_block(0, 0, slot)
                mm_block(0, 1, slot)

            for r in range(NR):
                g, c = divmod(r, NCH)
                if c == 0 and r > 0:
                    for slot in range(GN):
                        saccs[slot] = saccpool.tile([P, S], dt.float32,
                                                    name=f"sacc{slot}", tag=f"sacc{slot}", bufs=1)
                for sb in range(NSB):
                    if sb == 0:
                        issue_xt(r, 1)
                    elif sb == 2 and r < NR - 1:
                        issue_xt(r + 1, 0)
                    if not (r == 0 and sb < 2):
                        # passes (0,0)/(0,1) were issued in the diagonal prologue
                        mm_pass(r, sb)
                    if r < NR - 1:
                        deq_head(r + 1, sb)
                        if sb >= 1:
                            deq_body(sb - 1)
                            deq_finish(sb - 1)
                        if sb == NSB - 1:
                            deq_body(3)
                            deq_finish(3)
                if c == NCH - 1:
                    for sb in range(NSB):
                        for slot in range(GN):
                            ot = g * GN + slot
                            nc.sync.dma_start(
                                out=out_d[ot * P:(ot + 1) * P, sb * SBW:(sb + 1) * SBW],
                                in_=saccs[slot][:, sb * SBW:(sb + 1) * SBW])


_NC_CACHE = None


def _get_nc():
    global _NC_CACHE
    if _NC_CACHE is None:
        nc = bacc.Bacc("TRN2", num_devices=N_CORES)
        _build_kernel(nc)
        nc.compile()
        _NC_CACHE = nc
    return _NC_CACHE


def kernel(x, qweight, lut):
    x = np.asarray(x)
    qweight = np.asarray(qweight)
    lut = np.asarray(lut)
    xbf = np.ascontiguousarray(
        x.reshape(S, K).astype(np.float32, copy=False)).astype(ml_dtypes.bfloat16)

    in_maps = []
    for c in range(N_CORES):
        o0, o1 = c * O, (c + 1) * O
        qb = np.ascontiguousarray(qweight[o0:o1]).view(np.uint8).reshape(O, K)
        tb, dl, tw = _host_tables(lut[o0:o1])
        in_maps.append({"xbf": xbf, "qw": qb, "tb": tb, "dl": dl, "tw": tw})

    nc = _get_nc()
    res = bass_utils.run_bass_kernel_spmd(nc, in_maps, core_ids=list(range(N_CORES)))
    out_full = np.concatenate([res.results[c]["out"] for c in range(N_CORES)], axis=0)
    return np.ascontiguousarray(out_full.T).reshape(1, S, 8192).astype(np.float32, copy=False)
